# revision 49
# baseline (speedup 1.0000x reference)
"""Trainium2 Bass kernel for nn_LinearGML2.

Computes out[b, k] = || (x_b - w_k) @ L_k ||_2 for K=256 per-class
lower-triangular matrices L_k (diag = L_diags**2, strict lower = L_lower),
B=1024, d=512.  Sharded over classes: 32 per core.

Math: ||(x - w_k) L_k||^2 = ||x L_k||^2 - 2 x . g_k + c_k with
g_k = L_k L_k^T w_k^T, c_k = ||w_k L_k||^2 (host-precomputed vectors).
The big matmul's only per-class operand is L; x is class-independent, so
there is no per-class VectorE prep pass.

All matmul inputs are fp8e4 with DoubleRow perf mode (contract 256 rows
per instruction, HW ~1.44x over bf16): the 512x512 triangular L splits
into 2 d-superblocks x (2 or 4) e-blocks with 6 nonzero pairs.

The sum-of-squares epilogue is the wall (every psum element crosses one
engine at ~1 elem/cyc/partition, and TRN2 ops read at most ONE psum
operand), so it is spread over THREE sinks:
  * KT "transposed" classes, layout u^T[e, b]: ScalarE squares psum ->
    SBUF fp8 (no accumulator read), TensorE reduces over e via
    one-hot-lhsT DoubleRow matmuls accumulating ssq rows (plus the
    -2 x.g cross terms; c_k via the Sqrt bias).  Costs extra PE columns,
    sized to soak spare ScalarE capacity only.
  * The remaining classes in normal layout u[b, e], per-chunk psum
    [128, 512] tiles drained by either VectorE bn_stats (+ batched
    decode) or ScalarE Square+accum_out, chosen per-chunk to balance
    the two engines; cross terms ride the shared-x lhsT into a one-bank
    psum region; c_k via a host-broadcast tile.
"""

from contextlib import ExitStack

import numpy as np

import concourse.bass as bass  # noqa: F401  (import keeps bass registered)
import concourse.tile as tile
from concourse import bacc, mybir
from concourse._compat import with_exitstack
from concourse.alu_op_type import AluOpType
from concourse.bass_utils import run_bass_kernel_spmd

K_CLASSES = 256
D = 512
B = 1024
N_CORES = 8
KC = K_CLASSES // N_CORES  # classes per core = 32
P = 128
BH = B // 2  # b-half = 512
KT = 4  # transposed (ScalarE-square + PE-reduce) classes
NU = KC - KT  # normal-orientation classes
NCH = B // P  # chunks = 8

_FP8 = mybir.dt.float8e4
_F32 = mybir.dt.float32
_SQUARE = mybir.ActivationFunctionType.Square
_SQRT = mybir.ActivationFunctionType.Sqrt
_DR = mybir.MatmulPerfMode.DoubleRow

FP8_NP = mybir.dt.np(_FP8)

# per-u-chunk drain engine: VectorE bn_stats (0) vs ScalarE accum (1);
# ScalarE also carries the KT transposed classes, so it takes 3/8 here.
_ACT_PAT = (0, 1, 0, 0, 1, 0, 0, 1)


@with_exitstack
def _gml2_kernel(ctx: ExitStack, tc: "tile.TileContext", out1, out2, xq, lt0q, lt1q, gq, oh, cb, cb2):
    nc = tc.nc
    const = ctx.enter_context(tc.tile_pool(name="const", bufs=1))
    ltpool = ctx.enter_context(tc.tile_pool(name="lt", bufs=1))
    sqpool = ctx.enter_context(tc.tile_pool(name="sq", bufs=6))
    dcpool = ctx.enter_context(tc.tile_pool(name="dc", bufs=2))
    mmT = ctx.enter_context(tc.tile_pool(name="mmT", bufs=1, space="PSUM"))
    upp = ctx.enter_context(tc.tile_pool(name="up", bufs=4, space="PSUM"))
    accp = ctx.enter_context(tc.tile_pool(name="acc", bufs=1, space="PSUM"))
    crp = ctx.enter_context(tc.tile_pool(name="cr", bufs=1, space="PSUM"))

    xq_sb = const.tile([P, 2, 2, B], _FP8, name="xq_sb")
    gq_sb = const.tile([P, 2, 2, KC], _FP8, name="gq_sb")
    oh_sb = const.tile([P, 2, KT, KC], _FP8, name="oh_sb")
    cb_sb = const.tile([KT, 1], _F32, name="cb_sb")
    cb2_sb = const.tile([P, NCH, NU], _F32, name="cb2_sb")
    bnst = const.tile([P, NCH, NU, 6], _F32, name="bnst")
    sqd = const.tile([P, NCH, NU], _F32, name="sqd")
    osb1 = const.tile([KT, 2, BH], _F32, name="osb1")
    osb2 = const.tile([P, NCH, NU], _F32, name="osb2")

    # x lands first (mains + cross need it), split across both big DMA
    # queues; the small constants ride the otherwise-idle Scalar queue.
    nc.vector.memset(bnst[:, :, :, :], 0.0)
    nc.sync.dma_start(xq_sb[:, 1, 0, :], xq[1, 0])
    nc.gpsimd.dma_start(xq_sb[:, 1, 1, :], xq[1, 1])
    nc.sync.dma_start(xq_sb[:, 0, 0, :], xq[0, 0])
    nc.gpsimd.dma_start(xq_sb[:, 0, 1, :], xq[0, 1])
    nc.scalar.dma_start(gq_sb[:, :, :, :], gq.rearrange("j s p m -> p j s m"))
    nc.scalar.dma_start(oh_sb[:, :, :, :], oh)
    nc.scalar.dma_start(cb_sb[:, :], cb)
    nc.scalar.dma_start(cb2_sb[:, :, :], cb2)

    lt0 = [None] * KC
    lt1 = [None] * KC

    def _load_lt(k, q, q2):
        lt0[k] = ltpool.tile([P, 2, 256], _FP8, tag=f"lt0_{k}", name=f"lt0_{k}")
        lt1[k] = ltpool.tile([P, 2, D], _FP8, tag=f"lt1_{k}", name=f"lt1_{k}")
        q.dma_start(lt1[k][:, :, :], lt1q[k])
        q2.dma_start(lt0[k][:, :, :], lt0q[k])

    # DMA in consumption order (u-classes KT.. with t-classes woven in)
    t_at = {max(0, (4 * t * NU) // (8 * KT) - 1): t for t in range(KT)}
    order = []
    for i in range(NU):
        order.append(KT + i)
        if i in t_at:
            order.append(t_at[i])
    # ScalarE's DMA queue is idle during the ramp: it carries the big
    # half of the first few classes so compute starts sooner.
    for i, k in enumerate(order):
        a, b = (nc.sync, nc.gpsimd) if i % 2 == 0 else (nc.gpsimd, nc.sync)
        if i < 6:
            a = nc.scalar if i % 2 == 1 else a
        _load_lt(k, a, b)

    creg = crp.tile([P, NCH, NU], _F32, name="creg")
    acc_box = {}

    def _get_acc(h):
        # lazily open acc[h]'s accumulation group with the cross matmuls:
        # acc[h][k, b] = -2 sum_d x g_k.  One bank, reused h=0 then h=1.
        if h not in acc_box:
            a = accp.tile([KC, BH], _F32, tag="acc", name=f"acc{h}")
            for j in (0, 1):
                nc.tensor.matmul(
                    a[:, :],
                    gq_sb[:, j, :, :],
                    xq_sb[:, j, :, h * BH : (h + 1) * BH],
                    start=(j == 0),
                    stop=False,
                    perf_mode=_DR,
                    skip_group_check=True,
                )
            acc_box[h] = a
        return acc_box[h]

    # ---- transposed-class units ----------------------------------------
    # One unit = (half, class, e-block-pair): mains into a 2-bank psum
    # tile + ScalarE fp8 square.  The PE one-hot reduce of a unit is
    # DEFERRED one schedule step so it never waits on its own drain.
    # All h=0 units come before h=1 so acc needs one bank at a time.
    T_UNITS = [
        (t, h, pair, mms)
        for h in (0, 1)
        for t in range(KT)
        for pair, mms in ((1, ((1, 2), (1, 3))), (0, ((1, 0), (1, 1), (0, 0), (0, 1))))
    ]

    def t_unit(u):
        t, h, pair, mms = T_UNITS[u]
        pt = mmT.tile([P, 2, BH], _F32, tag="mmT", name=f"mmT{t}_{h}{pair}")
        for j, c in mms:
            lsrc = lt1[t] if j == 1 else lt0[t]
            nc.tensor.matmul(
                pt[:, c % 2, :],
                lsrc[:, :, c * 128 : (c + 1) * 128],
                xq_sb[:, j, :, h * BH : (h + 1) * BH],
                start=(j == 1),
                stop=(j == 0) if c <= 1 else (j == 1),
                perf_mode=_DR,
            )
        s = sqpool.tile([P, 2, BH], _FP8, tag="sq", name=f"s{t}_{h}{pair}")
        nc.scalar.activation(s[:, :, :], pt[:, :, :], _SQUARE)
        return (u, t, h, s)

    def t_reduce(pending):
        u, t, h, s = pending
        last_of_h = u % (2 * KT) == 2 * KT - 1
        nc.tensor.matmul(
            _get_acc(h)[:, :],
            oh_sb[:, :, t, :],
            s,
            start=False,
            stop=last_of_h,
            perf_mode=_DR,
            skip_group_check=True,
        )
        if last_of_h:
            nc.scalar.activation(
                osb1[:, h, :], acc_box[h][0:KT, :], _SQRT, bias=cb_sb[:, :]
            )
            nc.sync.dma_start(out1[:, h, :], osb1[:, h, :])

    # ---- normal-orientation class emitter ------------------------------
    uchunk_cnt = [0]

    udrain_q = []

    def u_drain(up, ch, m):
        if _ACT_PAT[uchunk_cnt[0] % len(_ACT_PAT)]:
            # accumulated ssq lands in the (pre-zeroed) M2_e slot so the
            # uniform bn decode (M2e + M2o + 256(me^2+mo^2)) passes it
            # through unchanged.
            nc.scalar.activation(
                up[:, :], up[:, :], _SQUARE, accum_out=bnst[:, ch, m, 2:3]
            )
        else:
            nc.vector.bn_stats(bnst[:, ch, m, :], up[:, :])
        uchunk_cnt[0] += 1

    def u_class(k, kfirst):
        m = k - KT
        for ch in range(NCH):
            up = upp.tile([P, D], _F32, tag="up", name=f"up{k}_{ch}")
            for j in (1, 0):
                lhsT = xq_sb[:, j, :, ch * P : (ch + 1) * P]
                nc.tensor.matmul(
                    up[:, :] if j == 1 else up[:, 0:256],
                    lhsT,
                    lt1[k][:, :, :] if j == 1 else lt0[k][:, :, :],
                    start=(j == 1),
                    stop=(j == 0),
                    perf_mode=_DR,
                )
                if kfirst:
                    nc.tensor.matmul(
                        creg[:, ch, :],
                        lhsT,
                        gq_sb[:, j, :, KT:KC],
                        start=(ch == 0 and j == 1),
                        stop=(ch == NCH - 1 and j == 0),
                        perf_mode=_DR,
                        skip_group_check=True,
                    )
            # one-chunk lookahead: drain the PREVIOUS chunk so the drain
            # engine never waits on the matmul just issued.
            udrain_q.append((up, ch, m))
            if len(udrain_q) > 1:
                u_drain(*udrain_q.pop(0))

    def decode(ks):
        # sum(u^2) = M2_e + M2_o + 256 * (mean_e^2 + mean_o^2), only for
        # bn_stats chunks; ScalarE accum chunks already wrote sqd.
        me, m2e = bnst[:, :, ks, 1], bnst[:, :, ks, 2]
        mo, m2o = bnst[:, :, ks, 4], bnst[:, :, ks, 5]
        nk = len(range(*ks.indices(NU)))
        t1f = dcpool.tile([P, NCH, NU], _F32, tag="t1", name="t1")
        t2f = dcpool.tile([P, NCH, NU], _F32, tag="t2", name="t2")
        t1 = t1f[:, :, 0:nk]
        t2 = t2f[:, :, 0:nk]
        # runs on GpSimd: SBUF-only elementwise work, keeps VectorE free
        # for bn_stats.
        nc.gpsimd.tensor_mul(t1, me, me)
        nc.gpsimd.tensor_mul(t2, mo, mo)
        nc.gpsimd.tensor_add(t1, t1, t2)
        nc.gpsimd.tensor_add(t2, m2e, m2o)
        nc.gpsimd.tensor_scalar_mul(t1, t1, float(D // 2))
        nc.gpsimd.tensor_add(sqd[:, :, ks], t1, t2)

    # ---- schedule ------------------------------------------------------
    # one u-class per step; t-units spread evenly across steps, each
    # unit's PE reduce flushed on the following step.
    def final_combine(ks):
        # ssq_u = sqd - 2*cross + c (creg already holds -2 x.g).  The
        # psum read must stay on VectorE (GpSimd cannot touch PSUM).
        nk = len(range(*ks.indices(NU)))
        t1f = dcpool.tile([P, NCH, NU], _F32, tag="tf", name="t1f")
        t1 = t1f[:, :, 0:nk]
        nc.vector.scalar_tensor_tensor(
            t1, creg[:, :, ks], 1.0, cb2_sb[:, :, ks],
            AluOpType.mult, AluOpType.add,
        )
        nc.gpsimd.tensor_add(t1, t1, sqd[:, :, ks])
        nc.scalar.activation(osb2[:, :, ks], t1, _SQRT)
        nc.sync.dma_start(out2[:, :, ks], osb2[:, :, ks])

    NT = len(T_UNITS)
    pending = []
    emitted = 0
    done = 0
    for i in range(NU):
        u_class(KT + i, kfirst=(i == 0))
        while pending:
            t_reduce(pending.pop(0))
        want = ((i + 1) * NT) // NU
        while emitted < want:
            pending.append(t_unit(emitted))
            emitted += 1
        if i % 7 == 6:
            while udrain_q:
                u_drain(*udrain_q.pop(0))
            decode(slice(done, i + 1))
            done = i + 1
            if done == 14:
                final_combine(slice(0, 14))
    while udrain_q:
        u_drain(*udrain_q.pop(0))
    while emitted < NT:
        pending.append(t_unit(emitted))
        emitted += 1
    while pending:
        t_reduce(pending.pop(0))
    if done < NU:
        decode(slice(done, NU))
    final_combine(slice(14, NU))




_CACHE: dict = {}


def build_nc():
    if "nc" in _CACHE:
        return _CACHE["nc"]
    nc = bacc.Bacc("TRN2", target_bir_lowering=False, debug=False, num_devices=N_CORES)
    xq = nc.dram_tensor("xq", [2, 2, P, B], _FP8, kind="ExternalInput").ap()
    lt0q = nc.dram_tensor("lt0q", [KC, P, 2, 256], _FP8, kind="ExternalInput").ap()
    lt1q = nc.dram_tensor("lt1q", [KC, P, 2, D], _FP8, kind="ExternalInput").ap()
    gq = nc.dram_tensor("gq", [2, 2, P, KC], _FP8, kind="ExternalInput").ap()
    oh = nc.dram_tensor("oh", [P, 2, KT, KC], _FP8, kind="ExternalInput").ap()
    cb = nc.dram_tensor("cb", [KT, 1], _F32, kind="ExternalInput").ap()
    cb2 = nc.dram_tensor("cb2", [P, NCH, NU], _F32, kind="ExternalInput").ap()
    out1 = nc.dram_tensor("out1", [KT, 2, BH], _F32, kind="ExternalOutput").ap()
    out2 = nc.dram_tensor("out2", [P, NCH, NU], _F32, kind="ExternalOutput").ap()
    with tile.TileContext(nc) as tc:
        _gml2_kernel(tc, out1, out2, xq, lt0q, lt1q, gq, oh, cb, cb2)
    nc.compile()
    _CACHE["nc"] = nc
    return nc


def host_prep(inputs, weight, L_diags, L_lower):
    """Layout/dtype transforms + per-class g/c correction vectors."""
    x = np.asarray(inputs, dtype=np.float32)
    w = np.asarray(weight, dtype=np.float64).reshape(K_CLASSES, D)
    ld = np.asarray(L_diags, dtype=np.float64)
    ll = np.asarray(L_lower, dtype=np.float64)

    lmat = np.zeros((K_CLASSES, D, D), dtype=np.float64)
    ri, ci = np.tril_indices(D, k=-1)
    lmat[:, ri, ci] = ll
    dd = np.arange(D)
    lmat[:, dd, dd] = ld * ld

    # v_k = w_k L_k ; g_k = L_k v_k ; c_k = ||v_k||^2
    v = np.einsum("kd,kde->ke", w, lmat)
    g = np.einsum("kde,ke->kd", lmat, v)
    c = np.einsum("ke,ke->k", v, v)

    lmat32 = lmat.astype(np.float32)
    # weights element (p, s, m) = L[d = 256j + 128s + p, e]
    lt0q = np.ascontiguousarray(
        lmat32[:, 0:256, 0:256].reshape(K_CLASSES, 2, P, 256).transpose(0, 2, 1, 3)
    ).astype(FP8_NP)
    lt1q = np.ascontiguousarray(
        lmat32[:, 256:512, :].reshape(K_CLASSES, 2, P, D).transpose(0, 2, 1, 3)
    ).astype(FP8_NP)

    xq = np.ascontiguousarray(x.T.reshape(2, 2, P, B)).astype(FP8_NP)
    gT = (-2.0 * g).T.astype(np.float32).reshape(2, 2, P, K_CLASSES)  # (j, s, p, k)
    # one-hot column t among KC=32 (padded so the t-slice stride is 16-aligned)
    oh = np.broadcast_to(
        np.eye(KT, KC, dtype=np.float32)[None, None], (P, 2, KT, KC)
    )
    oh = np.ascontiguousarray(oh).astype(FP8_NP)
    return xq, lt0q, lt1q, gT, oh, c.astype(np.float32)


def make_in_maps(xq, lt0q, lt1q, gT, oh, c):
    in_maps = []
    for core in range(N_CORES):
        sl = slice(core * KC, (core + 1) * KC)
        cc = c[sl]
        cb2 = np.broadcast_to(cc[KT:][None, None, :], (P, NCH, NU))
        in_maps.append(
            {
                "xq": xq,
                "lt0q": np.ascontiguousarray(lt0q[sl]),
                "lt1q": np.ascontiguousarray(lt1q[sl]),
                "gq": np.ascontiguousarray(gT[:, :, :, sl]).astype(FP8_NP),
                "oh": oh,
                "cb": np.ascontiguousarray(cc[:KT].reshape(KT, 1)),
                "cb2": np.ascontiguousarray(cb2),
            }
        )
    return in_maps


def kernel(inputs, weight, L_diags, L_lower, **run_kwargs):
    packed = host_prep(inputs, weight, L_diags, L_lower)
    nc = build_nc()
    in_maps = make_in_maps(*packed)
    res = run_bass_kernel_spmd(nc, in_maps, core_ids=list(range(N_CORES)), **run_kwargs)
    out = np.empty((B, K_CLASSES), dtype=np.float32)
    for core in range(N_CORES):
        k0 = core * KC
        blk1 = np.asarray(res.results[core]["out1"]).astype(np.float32).reshape(KT, B)
        out[:, k0 : k0 + KT] = blk1.T
        blk2 = np.asarray(res.results[core]["out2"]).astype(np.float32)  # [P, NCH, NU]
        out[:, k0 + KT : k0 + KC] = blk2.transpose(1, 0, 2).reshape(B, NU)
    if run_kwargs:
        _CACHE["last_result"] = res
    return out


# revision 55
# speedup vs baseline: 1.0240x; 1.0240x over previous
"""Trainium2 Bass kernel for nn_LinearGML2.

Computes out[b, k] = || (x_b - w_k) @ L_k ||_2 for K=256 per-class
lower-triangular matrices L_k (diag = L_diags**2, strict lower = L_lower),
B=1024, d=512.  Sharded over classes: 32 per core.

Math: ||(x - w_k) L_k||^2 = ||x L_k||^2 - 2 x . g_k + c_k with
g_k = L_k L_k^T w_k^T, c_k = ||w_k L_k||^2 (host-precomputed vectors).
The big matmul's only per-class operand is L; x is class-independent, so
there is no per-class VectorE prep pass.

All matmul inputs are fp8e4 with DoubleRow perf mode (contract 256 rows
per instruction, HW ~1.44x over bf16): the 512x512 triangular L splits
into 2 d-superblocks x (2 or 4) e-blocks with 6 nonzero pairs.

The sum-of-squares epilogue is the wall (every psum element crosses one
engine at ~1 elem/cyc/partition, and TRN2 ops read at most ONE psum
operand), so it is spread over THREE sinks:
  * KT "transposed" classes, layout u^T[e, b]: ScalarE squares psum ->
    SBUF fp8 (no accumulator read), TensorE reduces over e via
    one-hot-lhsT DoubleRow matmuls accumulating ssq rows (plus the
    -2 x.g cross terms; c_k via the Sqrt bias).  Costs extra PE columns,
    sized to soak spare ScalarE capacity only.
  * The remaining classes in normal layout u[b, e], per-chunk psum
    [128, 512] tiles drained by either VectorE bn_stats (+ batched
    decode) or ScalarE Square+accum_out, chosen per-chunk to balance
    the two engines; cross terms ride the shared-x lhsT into a one-bank
    psum region; c_k via a host-broadcast tile.
"""

from contextlib import ExitStack

import numpy as np

import concourse.bass as bass  # noqa: F401  (import keeps bass registered)
import concourse.tile as tile
from concourse import bacc, mybir
from concourse._compat import with_exitstack
from concourse.alu_op_type import AluOpType
from concourse.bass_utils import run_bass_kernel_spmd

K_CLASSES = 256
D = 512
B = 1024
N_CORES = 8
KC = K_CLASSES // N_CORES  # classes per core = 32
P = 128
BH = B // 2  # b-half = 512
KT = 4  # transposed (ScalarE-square + PE-reduce) classes
NU = KC - KT  # normal-orientation classes
NCH = B // P  # chunks = 8

_FP8 = mybir.dt.float8e4
_F32 = mybir.dt.float32
_SQUARE = mybir.ActivationFunctionType.Square
_SQRT = mybir.ActivationFunctionType.Sqrt
_DR = mybir.MatmulPerfMode.DoubleRow

FP8_NP = mybir.dt.np(_FP8)

# per-u-chunk drain engine: VectorE bn_stats (0) vs ScalarE accum (1);
# ScalarE also carries the KT transposed classes, so it takes 3/8 here.
_ACT_PAT = (0, 1, 0, 0, 1, 0, 0, 1)


@with_exitstack
def _gml2_kernel(ctx: ExitStack, tc: "tile.TileContext", out1, out2, xq, lt0q, lt1q, gq, oh, cb, cb2):
    nc = tc.nc
    const = ctx.enter_context(tc.tile_pool(name="const", bufs=1))
    ltpool = ctx.enter_context(tc.tile_pool(name="lt", bufs=1))
    sqpool = ctx.enter_context(tc.tile_pool(name="sq", bufs=6))
    dcpool = ctx.enter_context(tc.tile_pool(name="dc", bufs=2))
    mmT = ctx.enter_context(tc.tile_pool(name="mmT", bufs=1, space="PSUM"))
    upp = ctx.enter_context(tc.tile_pool(name="up", bufs=4, space="PSUM"))
    accp = ctx.enter_context(tc.tile_pool(name="acc", bufs=1, space="PSUM"))
    crp = ctx.enter_context(tc.tile_pool(name="cr", bufs=1, space="PSUM"))

    xq_sb = const.tile([P, 2, 2, B], _FP8, name="xq_sb")
    gq_sb = const.tile([P, 2, 2, KC], _FP8, name="gq_sb")
    oh_sb = const.tile([P, 2, KT, KC], _FP8, name="oh_sb")
    cb_sb = const.tile([KT, 1], _F32, name="cb_sb")
    cb2_sb = const.tile([P, NCH, NU], _F32, name="cb2_sb")
    bnst = const.tile([P, NCH, NU, 6], _F32, name="bnst")
    sqd = const.tile([P, NCH, NU], _F32, name="sqd")
    osb1 = const.tile([KT, 2, BH], _F32, name="osb1")
    osb2 = const.tile([P, NCH, NU], _F32, name="osb2")

    nc.vector.memset(bnst[:, :, :, :], 0.0)
    for j in (1, 0):
        for s in (0, 1):
            nc.gpsimd.dma_start(xq_sb[:, j, s, :], xq[j, s])
    nc.gpsimd.dma_start(gq_sb[:, :, :, :], gq.rearrange("j s p m -> p j s m"))
    nc.gpsimd.dma_start(oh_sb[:, :, :, :], oh)
    nc.gpsimd.dma_start(cb_sb[:, :], cb)
    nc.gpsimd.dma_start(cb2_sb[:, :, :], cb2)

    lt0 = [None] * KC
    lt1 = [None] * KC

    def _load_lt(k, q):
        lt0[k] = ltpool.tile([P, 2, 256], _FP8, tag=f"lt0_{k}", name=f"lt0_{k}")
        lt1[k] = ltpool.tile([P, 2, D], _FP8, tag=f"lt1_{k}", name=f"lt1_{k}")
        q.dma_start(lt1[k][:, :, :], lt1q[k])
        q.dma_start(lt0[k][:, :, :], lt0q[k])

    # DMA in consumption order (u-classes KT.. with t-classes woven in)
    t_at = {max(0, (4 * t * NU) // (8 * KT) - 1): t for t in range(KT)}
    order = []
    for i in range(NU):
        order.append(KT + i)
        if i in t_at:
            order.append(t_at[i])
    # ScalarE is idle during the DMA ramp: let it carry every third load
    # for the first few classes so compute starts sooner.
    for i, k in enumerate(order):
        if i < 9:
            q = (nc.sync, nc.gpsimd, nc.scalar)[i % 3]
        else:
            q = nc.sync if i % 2 == 0 else nc.gpsimd
        _load_lt(k, q)

    creg = crp.tile([P, NCH, NU], _F32, name="creg")
    acc_box = {}

    def _get_acc(h):
        # lazily open acc[h]'s accumulation group with the cross matmuls:
        # acc[h][k, b] = -2 sum_d x g_k.  One bank, reused h=0 then h=1.
        if h not in acc_box:
            a = accp.tile([KC, BH], _F32, tag="acc", name=f"acc{h}")
            for j in (0, 1):
                nc.tensor.matmul(
                    a[:, :],
                    gq_sb[:, j, :, :],
                    xq_sb[:, j, :, h * BH : (h + 1) * BH],
                    start=(j == 0),
                    stop=False,
                    perf_mode=_DR,
                    skip_group_check=True,
                )
            acc_box[h] = a
        return acc_box[h]

    # ---- transposed-class units ----------------------------------------
    # One unit = (half, class, e-block-pair): mains into a 2-bank psum
    # tile + ScalarE fp8 square.  The PE one-hot reduce of a unit is
    # DEFERRED one schedule step so it never waits on its own drain.
    # All h=0 units come before h=1 so acc needs one bank at a time.
    T_UNITS = [
        (t, h, pair, mms)
        for h in (0, 1)
        for t in range(KT)
        for pair, mms in ((1, ((1, 2), (1, 3))), (0, ((1, 0), (1, 1), (0, 0), (0, 1))))
    ]

    def t_unit(u):
        t, h, pair, mms = T_UNITS[u]
        pt = mmT.tile([P, 2, BH], _F32, tag="mmT", name=f"mmT{t}_{h}{pair}")
        for j, c in mms:
            lsrc = lt1[t] if j == 1 else lt0[t]
            nc.tensor.matmul(
                pt[:, c % 2, :],
                lsrc[:, :, c * 128 : (c + 1) * 128],
                xq_sb[:, j, :, h * BH : (h + 1) * BH],
                start=(j == 1),
                stop=(j == 0) if c <= 1 else (j == 1),
                perf_mode=_DR,
            )
        s = sqpool.tile([P, 2, BH], _FP8, tag="sq", name=f"s{t}_{h}{pair}")
        nc.scalar.activation(s[:, :, :], pt[:, :, :], _SQUARE)
        return (u, t, h, s)

    def t_reduce(pending):
        u, t, h, s = pending
        last_of_h = u % (2 * KT) == 2 * KT - 1
        nc.tensor.matmul(
            _get_acc(h)[:, :],
            oh_sb[:, :, t, :],
            s,
            start=False,
            stop=last_of_h,
            perf_mode=_DR,
            skip_group_check=True,
        )
        if last_of_h:
            nc.scalar.activation(
                osb1[:, h, :], acc_box[h][0:KT, :], _SQRT, bias=cb_sb[:, :]
            )

    # ---- normal-orientation class emitter ------------------------------
    uchunk_cnt = [0]

    udrain_q = []

    def u_drain(up, ch, m):
        if _ACT_PAT[uchunk_cnt[0] % len(_ACT_PAT)]:
            # accumulated ssq lands in the (pre-zeroed) M2_e slot so the
            # uniform bn decode (M2e + M2o + 256(me^2+mo^2)) passes it
            # through unchanged.
            nc.scalar.activation(
                up[:, :], up[:, :], _SQUARE, accum_out=bnst[:, ch, m, 2:3]
            )
        else:
            nc.vector.bn_stats(bnst[:, ch, m, :], up[:, :])
        uchunk_cnt[0] += 1

    def u_class(k, kfirst):
        m = k - KT
        for ch in range(NCH):
            up = upp.tile([P, D], _F32, tag="up", name=f"up{k}_{ch}")
            for j in (1, 0):
                lhsT = xq_sb[:, j, :, ch * P : (ch + 1) * P]
                nc.tensor.matmul(
                    up[:, :] if j == 1 else up[:, 0:256],
                    lhsT,
                    lt1[k][:, :, :] if j == 1 else lt0[k][:, :, :],
                    start=(j == 1),
                    stop=(j == 0),
                    perf_mode=_DR,
                )
                if kfirst:
                    nc.tensor.matmul(
                        creg[:, ch, :],
                        lhsT,
                        gq_sb[:, j, :, KT:KC],
                        start=(ch == 0 and j == 1),
                        stop=(ch == NCH - 1 and j == 0),
                        perf_mode=_DR,
                        skip_group_check=True,
                    )
            u_drain(up, ch, m)

    def decode(ks):
        # sum(u^2) = M2_e + M2_o + 256 * (mean_e^2 + mean_o^2), only for
        # bn_stats chunks; ScalarE accum chunks already wrote sqd.
        me, m2e = bnst[:, :, ks, 1], bnst[:, :, ks, 2]
        mo, m2o = bnst[:, :, ks, 4], bnst[:, :, ks, 5]
        nk = len(range(*ks.indices(NU)))
        t1f = dcpool.tile([P, NCH, NU], _F32, tag="t1", name="t1")
        t2f = dcpool.tile([P, NCH, NU], _F32, tag="t2", name="t2")
        t1 = t1f[:, :, 0:nk]
        t2 = t2f[:, :, 0:nk]
        # runs on GpSimd: SBUF-only elementwise work, keeps VectorE free
        # for bn_stats.
        nc.gpsimd.tensor_mul(t1, me, me)
        nc.gpsimd.tensor_mul(t2, mo, mo)
        nc.gpsimd.tensor_add(t1, t1, t2)
        nc.gpsimd.tensor_add(t2, m2e, m2o)
        nc.gpsimd.tensor_scalar_mul(t1, t1, float(D // 2))
        nc.gpsimd.tensor_add(sqd[:, :, ks], t1, t2)

    # ---- schedule ------------------------------------------------------
    # one u-class per step; t-units spread evenly across steps, each
    # unit's PE reduce flushed on the following step.
    def final_combine(ks):
        # ssq_u = sqd - 2*cross + c (creg already holds -2 x.g).  The
        # psum read must stay on VectorE (GpSimd cannot touch PSUM).
        nk = len(range(*ks.indices(NU)))
        t1f = dcpool.tile([P, NCH, NU], _F32, tag="tf", name="t1f")
        t1 = t1f[:, :, 0:nk]
        nc.vector.scalar_tensor_tensor(
            t1, creg[:, :, ks], 1.0, cb2_sb[:, :, ks],
            AluOpType.mult, AluOpType.add,
        )
        nc.gpsimd.tensor_add(t1, t1, sqd[:, :, ks])
        nc.scalar.activation(osb2[:, :, ks], t1, _SQRT)
        nc.sync.dma_start(out2[:, :, ks], osb2[:, :, ks])

    NT = len(T_UNITS)
    pending = []
    emitted = 0
    done = 0
    for i in range(NU):
        u_class(KT + i, kfirst=(i == 0))
        while pending:
            t_reduce(pending.pop(0))
        want = ((i + 1) * NT) // NU
        while emitted < want:
            pending.append(t_unit(emitted))
            emitted += 1
        if i % 7 == 6:
            decode(slice(done, i + 1))
            done = i + 1
    while pending:
        t_reduce(pending.pop(0))
    if done < NU:
        decode(slice(done, NU))
    final_combine(slice(0, NU))
    nc.sync.dma_start(out1, osb1[:, :, :])




_CACHE: dict = {}


def build_nc():
    if "nc" in _CACHE:
        return _CACHE["nc"]
    nc = bacc.Bacc("TRN2", target_bir_lowering=False, debug=False, num_devices=N_CORES)
    xq = nc.dram_tensor("xq", [2, 2, P, B], _FP8, kind="ExternalInput").ap()
    lt0q = nc.dram_tensor("lt0q", [KC, P, 2, 256], _FP8, kind="ExternalInput").ap()
    lt1q = nc.dram_tensor("lt1q", [KC, P, 2, D], _FP8, kind="ExternalInput").ap()
    gq = nc.dram_tensor("gq", [2, 2, P, KC], _FP8, kind="ExternalInput").ap()
    oh = nc.dram_tensor("oh", [P, 2, KT, KC], _FP8, kind="ExternalInput").ap()
    cb = nc.dram_tensor("cb", [KT, 1], _F32, kind="ExternalInput").ap()
    cb2 = nc.dram_tensor("cb2", [P, NCH, NU], _F32, kind="ExternalInput").ap()
    out1 = nc.dram_tensor("out1", [KT, 2, BH], _F32, kind="ExternalOutput").ap()
    out2 = nc.dram_tensor("out2", [P, NCH, NU], _F32, kind="ExternalOutput").ap()
    with tile.TileContext(nc) as tc:
        _gml2_kernel(tc, out1, out2, xq, lt0q, lt1q, gq, oh, cb, cb2)
    nc.compile()
    _CACHE["nc"] = nc
    return nc


def host_prep(inputs, weight, L_diags, L_lower):
    """Layout/dtype transforms + per-class g/c correction vectors."""
    x = np.asarray(inputs, dtype=np.float32)
    w = np.asarray(weight, dtype=np.float64).reshape(K_CLASSES, D)
    ld = np.asarray(L_diags, dtype=np.float64)
    ll = np.asarray(L_lower, dtype=np.float64)

    lmat = np.zeros((K_CLASSES, D, D), dtype=np.float64)
    ri, ci = np.tril_indices(D, k=-1)
    lmat[:, ri, ci] = ll
    dd = np.arange(D)
    lmat[:, dd, dd] = ld * ld

    # v_k = w_k L_k ; g_k = L_k v_k ; c_k = ||v_k||^2
    v = np.einsum("kd,kde->ke", w, lmat)
    g = np.einsum("kde,ke->kd", lmat, v)
    c = np.einsum("ke,ke->k", v, v)

    lmat32 = lmat.astype(np.float32)
    # weights element (p, s, m) = L[d = 256j + 128s + p, e]
    lt0q = np.ascontiguousarray(
        lmat32[:, 0:256, 0:256].reshape(K_CLASSES, 2, P, 256).transpose(0, 2, 1, 3)
    ).astype(FP8_NP)
    lt1q = np.ascontiguousarray(
        lmat32[:, 256:512, :].reshape(K_CLASSES, 2, P, D).transpose(0, 2, 1, 3)
    ).astype(FP8_NP)

    xq = np.ascontiguousarray(x.T.reshape(2, 2, P, B)).astype(FP8_NP)
    gT = (-2.0 * g).T.astype(np.float32).reshape(2, 2, P, K_CLASSES)  # (j, s, p, k)
    # one-hot column t among KC=32 (padded so the t-slice stride is 16-aligned)
    oh = np.broadcast_to(
        np.eye(KT, KC, dtype=np.float32)[None, None], (P, 2, KT, KC)
    )
    oh = np.ascontiguousarray(oh).astype(FP8_NP)
    return xq, lt0q, lt1q, gT, oh, c.astype(np.float32)


def make_in_maps(xq, lt0q, lt1q, gT, oh, c):
    in_maps = []
    for core in range(N_CORES):
        sl = slice(core * KC, (core + 1) * KC)
        cc = c[sl]
        cb2 = np.broadcast_to(cc[KT:][None, None, :], (P, NCH, NU))
        in_maps.append(
            {
                "xq": xq,
                "lt0q": np.ascontiguousarray(lt0q[sl]),
                "lt1q": np.ascontiguousarray(lt1q[sl]),
                "gq": np.ascontiguousarray(gT[:, :, :, sl]).astype(FP8_NP),
                "oh": oh,
                "cb": np.ascontiguousarray(cc[:KT].reshape(KT, 1)),
                "cb2": np.ascontiguousarray(cb2),
            }
        )
    return in_maps


def kernel(inputs, weight, L_diags, L_lower, **run_kwargs):
    packed = host_prep(inputs, weight, L_diags, L_lower)
    nc = build_nc()
    in_maps = make_in_maps(*packed)
    res = run_bass_kernel_spmd(nc, in_maps, core_ids=list(range(N_CORES)), **run_kwargs)
    out = np.empty((B, K_CLASSES), dtype=np.float32)
    for core in range(N_CORES):
        k0 = core * KC
        blk1 = np.asarray(res.results[core]["out1"]).astype(np.float32).reshape(KT, B)
        out[:, k0 : k0 + KT] = blk1.T
        blk2 = np.asarray(res.results[core]["out2"]).astype(np.float32)  # [P, NCH, NU]
        out[:, k0 + KT : k0 + KC] = blk2.transpose(1, 0, 2).reshape(B, NU)
    if run_kwargs:
        _CACHE["last_result"] = res
    return out
